# revision 40
# baseline (speedup 1.0000x reference)
"""Trainium2 Bass kernel for nn_DiversityLoss (cosine diversity loss).

Math: for each sample b with length L_b, the reference computes
    S = Xn @ Xn.T  (Xn = row-normalized, padding rows zeroed)
    sum_off[b] = sum(S) - L_b
    per_sample[b] = sum_off[b] / (L_b*(L_b-1))  if L_b > 1 else 0
    out = sum(per_sample) / count(L_b != 1)

Key identity: sum(S) over the valid block equals ||sum_t xn_t||^2, so the
device only needs, per sample, v_b = sum over valid rows of x_t/||x_t||
(a length-D vector). The O(T^2) Gram matrix is never materialized.

Sharding: valid rows are tiled into 128-row sample-aligned tiles; the tiles
are distributed evenly over the 8 cores (balanced by actual row count, per
the data-parallel hint but load-balanced over the ragged lengths). Each core
computes z[g] = sum_p r[p,g] * x[p,g,:] per tile g via the tensor engine
(r = reciprocal row norms). The host reduces the per-tile partial sums into
per-sample vectors and applies the closed-form scalar epilogue (the
"all-reduce of the scalar numerator" from the hint).

v2: the input ships as bf16 (host-side cast is free; halves HBM traffic and
lets the PE read the DMA'd buffer directly), split into 4 DMA chunks issued
from 4 different engine queues so descriptor generation is parallel. Per
chunk: ACT square (bf16->f32) -> DVE grouped reduce + reciprocal -> ACT
sqrt (bf16 r) -> PE per-group weighted-sum matmul -> Pool engine copies the
psum column block to SBUF. A single DMA returns z to DRAM.
"""

import math
import os
from contextlib import ExitStack

import numpy as np
import ml_dtypes

import concourse.bass as bass
import concourse.bacc as bacc
from concourse import mybir
from concourse.bass_utils import run_bass_kernel_spmd

N_CORES = 8
P = 128  # rows per tile == SBUF partitions
D = 64   # feature dim (hardcoded for this problem)

# 0: no stripping; 1: drop unused const-ap memsets; 2: + drop the initial
# bass all-engine barrier (safe: every body dependency is covered by
# explicit kernel semaphores, and the const-ap memset on Pool retires
# microseconds before the first ACT activation reads the bias);
# 3: + drop the exit-block drains/aeb barrier (the walrus postamble
# rendezvouses all engines and drains DMA queues itself, so the bass-level
# copies are redundant; measured ~0.8us faster)
STRIP_LEVEL = int(os.environ.get("KSTRIP", "3"))

_NC_CACHE: dict[tuple, bass.Bass] = {}


def _ver_pad() -> int:
    """Columns of output padding that encode a hash of this source file.

    The PJRT/axon compile path caches executables keyed on the HLO alone,
    and the bass program is carried out-of-band — two revisions of this
    kernel with identical tensor shapes silently reuse whichever NEFF was
    compiled first. Padding the output shape with a source-derived count
    forces a distinct HLO (and thus a fresh compile) for every revision.
    """
    import zlib

    try:
        with open(__file__, "rb") as f:
            h = zlib.crc32(f.read() + str(STRIP_LEVEL).encode())
    except Exception:
        h = STRIP_LEVEL
    return h % 251 + 1


def _chunk_bounds(G: int):
    """Split [0, G) into up to 4 chunks. Front chunks are issued first and
    should carry the bulk; the final chunk is kept small so the tail of the
    pipeline (square -> reduce -> recip -> sqrt -> matmul) is short."""
    if G <= 4:
        return [(g, g + 1) for g in range(G)]
    # Interior boundaries must be multiples of 4 groups: the DVE Reciprocal
    # writes in 16-byte granules, and an unaligned range start clobbers the
    # preceding groups of the same f32 row (observed as NaN in exactly the
    # last-two groups of every unaligned chunk). 4 f32 groups = 16 bytes;
    # this also keeps the bf16 rbf writes word-aligned. Chunk 0 carries the
    # bulk: it rides the earliest-issuing queue (ACT).
    c3 = ((G - 1) // 4) * 4
    cuts = [c for c in (4, c3 - 4, c3) if 0 < c < G]
    cuts = sorted(set(cuts))
    bounds = []
    g0 = 0
    for c in cuts:
        if c > g0:
            bounds.append((g0, c))
            g0 = c
    if g0 < G:
        bounds.append((g0, G))
    return bounds


def _build_nc_raw(G: int) -> bass.Bass:
    """Raw-Bass (hand-semaphored): no TileContext. Every cross-engine
    dependency is an explicit standalone wait."""
    nc = bacc.Bacc()
    f32 = mybir.dt.float32
    bf16 = mybir.dt.bfloat16
    xp = nc.dram_tensor("xp", [P, G * D], bf16, kind="ExternalInput")
    zo = nc.dram_tensor("z", [D, G + _ver_pad()], f32, kind="ExternalOutput")
    bounds = _chunk_bounds(G)
    C = len(bounds)
    # one DMA instruction per chunk; HWDGE exists only on SP/Activation,
    # gpsimd has the (slow, ~1us-later-completing) SWDGE queue. Chunks 0
    # and 1 (the bulk, on the critical square->reduce chain) both ride the
    # ACT queue, which enters the body earliest — the walrus preamble holds
    # SP back ~700ns and SWDGE completions trail HWDGE by ~1us. The ACT
    # table load auto-sinks below both issues (see _dedup_act_loads).
    issuers = (["scalar", "scalar", "sync", "gpsimd"] * ((C + 3) // 4))[:C]

    with ExitStack() as ctx:
        en = ctx.enter_context
        xall = en(nc.sbuf_tensor("xall", [P, G * D], bf16))
        sqall = en(nc.sbuf_tensor("sqall", [P, G * D], f32))
        ss = en(nc.sbuf_tensor("ss", [P, G], f32))
        iss = en(nc.sbuf_tensor("iss", [P, G], f32))
        rbf = en(nc.sbuf_tensor("rbf", [P, G], bf16))
        zsb = en(nc.sbuf_tensor("zsb", [D, G], f32))
        pz = en(nc.psum_tensor("pz", [D, G], f32))
        dma_sems = [en(nc.semaphore(f"dma_sem{i}")) for i in range(C)]
        sq_sem = en(nc.semaphore("sq_sem"))    # ACT square done (per chunk)
        rd_sem = en(nc.semaphore("rd_sem"))    # DVE reduce done (per chunk)
        rr_sem = en(nc.semaphore("rr_sem"))    # DVE recip done (per chunk)
        rb_sem = en(nc.semaphore("rb_sem"))    # ACT sqrt -> rbf done (per chunk)
        pe_sem = en(nc.semaphore("pe_sem"))    # PE matmuls done (per chunk)
        cp_sem = en(nc.semaphore("cp_sem"))    # ACT psum->sbuf copy done
        out_sem = en(nc.semaphore("out_sem"))  # output DMA completion

        # recip/sqrt run once per HALF (fixed per-instruction cost dominates
        # these tiny ops): halves = chunks [0..mid) and [mid..C)
        mid = (C + 1) // 2
        halves = [h for h in (bounds[:mid], bounds[mid:]) if h]
        mranges = [(h[0][0], h[-1][1]) for h in halves]
        # chunk count at the end of each half, for rd_sem waits
        mcounts = [mid, C] if len(halves) == 2 else [C]

        with nc.Block(no_gpsimd_drain=True) as block:

            def issue_dma(eng, name):
                for i, (g0, g1) in enumerate(bounds):
                    if issuers[i] != name:
                        continue
                    eng.dma_start(
                        out=xall[:, g0 * D : g1 * D], in_=xp[:, g0 * D : g1 * D]
                    ).then_inc(dma_sems[i], 16)

            @block.sync
            def _(sync):
                issue_dma(sync, "sync")
                sync.wait_ge(cp_sem, 1)
                sync.dma_start(out=zo[:, 0:G], in_=zsb[:, :]).then_inc(out_sem, 16)

            @block.gpsimd
            def _(gpsimd):
                issue_dma(gpsimd, "gpsimd")

            @block.scalar
            def _(scalar):
                issue_dma(scalar, "scalar")

                for ci, (g0, g1) in enumerate(bounds):
                    scalar.wait_ge(dma_sems[ci], 16)
                    scalar.activation(
                        sqall[:, g0 * D : g1 * D],
                        xall[:, g0 * D : g1 * D],
                        mybir.ActivationFunctionType.Square,
                    ).then_inc(sq_sem, 1)

                for hi, (g0, g1) in enumerate(mranges):
                    scalar.wait_ge(rr_sem, hi + 1)
                    with nc.allow_low_precision(
                        reason="bf16 r for the PE weighted-sum; norms stay f32"
                    ):
                        scalar.activation(
                            rbf[:, g0:g1],
                            iss[:, g0:g1],
                            mybir.ActivationFunctionType.Sqrt,
                        ).then_inc(rb_sem, 1)

                # single psum->sbuf copy once the PE pipeline has drained
                # (GPSIMD cannot read PSUM; ACT is idle by this point)
                scalar.wait_ge(pe_sem, 1)
                scalar.activation(
                    zsb[:, :], pz[:, :], mybir.ActivationFunctionType.Copy
                ).then_inc(cp_sem, 1)

            @block.vector
            def _(vector):
                hi = 0
                for ci, (g0, g1) in enumerate(bounds):
                    vector.wait_ge(sq_sem, ci + 1)
                    vector.reduce_sum(
                        ss[:, g0:g1],
                        sqall[:, g0 * D : g1 * D].rearrange(
                            "p (g d) -> p g d", d=D
                        ),
                        axis=mybir.AxisListType.X,
                    ).then_inc(rd_sem, 1)
                    if hi < len(mranges) and ci + 1 == mcounts[hi]:
                        m0, m1 = mranges[hi]
                        # the DVE pipelines queued instructions: without
                        # this wait the reciprocal reads ss while a reduce
                        # is still writing it (same-engine RAW hazard)
                        vector.wait_ge(rd_sem, ci + 1)
                        vector.reciprocal(iss[:, m0:m1], ss[:, m0:m1]).then_inc(
                            rr_sem, 1
                        )
                        hi += 1

            @block.tensor
            def _(tensor):
                for hi, (g0, g1) in enumerate(mranges):
                    tensor.wait_ge(rb_sem, hi + 1)
                    for g in range(g0, g1):
                        tensor.matmul(
                            pz[:, g : g + 1],
                            lhsT=xall[:, g * D : (g + 1) * D],
                            rhs=rbf[:, g : g + 1],
                            start=True,
                            stop=True,
                        )
                # drain the PE pipeline before signalling the psum->sbuf
                # copy: the final matmuls' PSUM writes lag their sequencer
                # retire, and the ACT copy otherwise reads uncommitted psum
                tensor.drain().then_inc(pe_sem, 1)

    nc.compile()
    _dedup_act_loads(nc)
    if STRIP_LEVEL > 0:
        _strip_preamble(nc, STRIP_LEVEL)
    return nc


def _dedup_act_loads(nc) -> None:
    """Bacc inserts one ACT table load per activation family (Square and
    Sqrt live in different default sets). One set (sqrt_and_friends)
    contains both functions plus Copy, so retarget the first load and drop
    the rest. The pass hoists the load to the top of the ACT block — ahead
    of the chunk-0 DMA issue on the ACT queue — so also sink it to just
    before the first activation (the load overlaps the DMA wait there)."""
    from concourse.hw_specs import get_activation_tables

    sqrt_set_id = list(get_activation_tables(nc.m.arch).keys()).index(
        "sqrt_and_friends"
    )
    seen = None
    for func in nc.m.functions:
        for blk in func.blocks:
            insts = blk.instructions
            keep = []
            changed = False
            for inst in insts:
                if isinstance(inst, mybir.InstLoadActFuncSet):
                    if seen is None:
                        inst.act_func_set_id = sqrt_set_id
                        seen = inst
                        keep.append(inst)
                    else:
                        changed = True
                        continue
                else:
                    keep.append(inst)
            if changed:
                blk.instructions = keep
    # sink the surviving load to just before the first activation
    for func in nc.m.functions:
        for blk in func.blocks:
            insts = blk.instructions
            if seen not in insts:
                continue
            li = insts.index(seen)
            ai = next(
                (
                    i
                    for i, inst in enumerate(insts)
                    if isinstance(inst, mybir.InstActivation)
                ),
                None,
            )
            if ai is not None and ai > li + 1:
                insts.pop(li)
                insts.insert(ai - 1, seen)
                blk.instructions = insts


def _strip_preamble(nc, level: int) -> None:
    """Remove framework-emitted start-up/teardown work this kernel does not
    need. The entry block holds the const-ap memsets plus the initial
    all-engine barrier; the exit block holds per-engine drains plus a
    sem-only all-engine barrier. The walrus-level preamble/postamble that
    wraps the bass program has its own engine rendezvous and final drains,
    and every cross-engine dependency in the body is covered by explicit
    kernel semaphores, so these bass-level barriers are redundant.

    level>=1: drop the unused const-ap memsets (only const-f32-0.0 is read,
      as the implicit activation bias).
    level>=2: drop the entry-block barrier (drains + barrier_* sems).
    level>=3: drop the exit-block drains + aeb_* sems as well.
    """
    func = nc.m.functions[0]
    entry, exit_blk = func.blocks[0], func.blocks[-1]

    keep = []
    for inst in entry.instructions:
        if level >= 1 and isinstance(inst, mybir.InstMemset):
            memref = str(getattr(inst.outs[0], "memref", ""))
            if memref.startswith("const-") and memref != "const-float32-0.0":
                continue
        if level >= 2 and isinstance(
            inst, (mybir.InstDrain, mybir.InstEventSemaphore)
        ):
            continue
        keep.append(inst)
    entry.instructions = keep

    if level >= 3 and exit_blk is not entry:
        exit_blk.instructions = [
            i
            for i in exit_blk.instructions
            if not isinstance(i, (mybir.InstDrain, mybir.InstEventSemaphore))
        ]


def _get_nc(G: int) -> bass.Bass:
    key = (G, STRIP_LEVEL)
    if key not in _NC_CACHE:
        _NC_CACHE[key] = _build_nc_raw(G)
    return _NC_CACHE[key]


def _pack_inputs(target: np.ndarray, lens: np.ndarray):
    """Tile valid rows into 128-row sample-aligned tiles, balance over cores,
    and lay each core's tiles out partition-major ([128, G*64]) in bf16."""
    B, T, Dd = target.shape
    assert Dd == D
    tiles = []  # (sample, t0, nrows)
    for b in range(B):
        L = int(lens[b])
        for t0 in range(0, L, P):
            tiles.append((b, t0, min(P, L - t0)))
    NT = len(tiles)
    G = max(1, math.ceil(NT / N_CORES))
    tgt_bf = target.astype(ml_dtypes.bfloat16)
    xps, gmaps, pads = [], [], []
    for c in range(N_CORES):
        sub = tiles[c * G : (c + 1) * G]
        # Padding rows are e0 = (1,0,...,0): unit norm, so the kernel (which
        # computes r = sqrt(1/ss) with NO epsilon) sees ss=1 and each pad row
        # contributes exactly e0 to its group sum; the host subtracts the
        # known pad counts afterwards. Avoids inf/NaN from all-zero rows.
        buf = np.zeros((G, P, D), dtype=ml_dtypes.bfloat16)
        buf[:, :, 0] = 1.0
        gmap = np.full((G,), -1, dtype=np.int64)
        pad = np.full((G,), P, dtype=np.int64)
        for g, (b, t0, rows) in enumerate(sub):
            buf[g, :rows, :] = tgt_bf[b, t0 : t0 + rows, :]
            gmap[g] = b
            pad[g] = P - rows
        xps.append(np.ascontiguousarray(buf.transpose(1, 0, 2)).reshape(P, G * D))
        gmaps.append(gmap)
        pads.append(pad)
    return xps, gmaps, pads, G


def kernel(target: np.ndarray, target_len: np.ndarray, _run_kwargs=None):
    target = np.asarray(target, dtype=np.float32)
    lens = np.asarray(target_len)
    B = target.shape[0]

    xps, gmaps, pads, G = _pack_inputs(target, lens)
    nc = _get_nc(G)

    in_maps = [{"xp": xps[c]} for c in range(N_CORES)]
    res = run_bass_kernel_spmd(
        nc, in_maps, core_ids=list(range(N_CORES)), **(_run_kwargs or {})
    )
    if _run_kwargs is not None:
        _run_kwargs["_last_result"] = res

    # host epilogue: combine per-tile partials into per-sample vectors
    V = np.zeros((B, D), dtype=np.float64)
    for c in range(N_CORES):
        z = np.asarray(res.results[c]["z"], dtype=np.float64).T[:G]  # [G, 64]
        z[:, 0] -= pads[c]  # remove the e0 padding-row contributions
        gm = gmaps[c]
        for b in range(B):
            sel = gm == b
            if sel.any():
                V[b] += z[sel].sum(axis=0)

    lens_f = lens.astype(np.float64)
    ssb = (V * V).sum(axis=1)  # ||v_b||^2 == sum(S_b)
    sum_off = ssb - lens_f
    pair = np.where(lens_f > 1, lens_f * (lens_f - 1.0), 1.0)
    per_sample = np.where(lens_f > 1, sum_off / pair, 0.0)
    denom = float((lens_f != 1).sum())
    return np.asarray(per_sample.sum() / denom, dtype=np.float32)
